# revision 18
# baseline (speedup 1.0000x reference)
"""Trainium2 Bass kernel for nn_DGC_Attention (global-context attention block).

Math (per batch b):
    cm[s]   = sum_c x[b,c,s] * wm[c]            (+ bm, which cancels in softmax)
    mask[s] = softmax(cm)[s] + 1/S              (uniform part: softmax of zeros)
    ctx[c]  = sum_s x[b,c,s] * mask[s]
    t       = relu(LN(ctx @ w1.T + b1) * ln_g + ln_b)
    out     = t @ w2.T + b2                     -> [B, C, 1, 1]

Sharding: pure data parallel, batch dim (16) over 8 cores, 2 batches/core.

Key restructuring: no max-subtraction is needed (cm has small range), so
e = exp(cm) is computed per s-HALF and the context sum is assembled from
unnormalized partials:
    ctx[c] = (1/Z) * sum_h sum_{s in h} x[c,s] e[s]  +  (1/S) * sum_s x[c,s]
Z and the 1/S uniform part fold into the tiny MLP stage.  This removes the
global softmax barrier: each (batch, s-half) phase pipelines independently,
so the weighted-pool pass overlaps the DMA stream except for the last half.

Per-core pipeline, per (batch, s-half) phase (x stays fp32/f32r on chip):
    - 8 chunk DMAs [128, 2048] fp32 (HWDGE, 1 MB each).
    - PE: cm partial via f32r matmuls -> [1, 2048] PSUM (4 banks, 2 slots).
    - ACT per chunk: Copy with accum_out -> rowsum column (uniform part),
      runs as chunks arrive (fully hidden).
    - ACT at half end: one Exp over PSUM row, accum_out = Z half.
    - GPSIMD: partition_broadcast e -> eB [128, 2048].
    - DVE per chunk: scalar_tensor_tensor (x * eB, accum) -> ctx_e column.
    - MLP tail: combine halves, scale by 1/Z (via partition-broadcast scalar)
      and 1/S, then matmuls + LayerNorm in transposed layout ([64, 2], LN via
      gpsimd partition_all_reduce); w1/w2 pre-transposed on host.
"""
import numpy as np

B_PER_CORE = 2
N_CORES = 8
C = 1024
S = 4096
H = 2                       # s-halves
SH = S // H                 # 2048
R = 64
NCHUNK = C // 128           # 8 c-chunks
LN_EPS = 1e-5

_CACHE = {}


def _build():
    import concourse.bass as bass
    import concourse.tile as tile
    from concourse import bacc, mybir, bass_isa

    f32 = mybir.dt.float32
    f32r = mybir.dt.float32r
    bf16 = mybir.dt.bfloat16
    AF = mybir.ActivationFunctionType
    ALU = mybir.AluOpType

    nc = bacc.Bacc("TRN2", target_bir_lowering=False, debug=False, num_devices=N_CORES)

    x_d = nc.dram_tensor("x", [B_PER_CORE, C, S], f32, kind="ExternalInput").ap()
    wmT_d = nc.dram_tensor("wmT", [128, NCHUNK], f32, kind="ExternalInput").ap()
    w1t_d = nc.dram_tensor("w1t", [128, NCHUNK * R], f32, kind="ExternalInput").ap()
    w2t_d = nc.dram_tensor("w2t", [R, C], f32, kind="ExternalInput").ap()
    b1_d = nc.dram_tensor("b1c", [R, 1], f32, kind="ExternalInput").ap()
    lng_d = nc.dram_tensor("lngc", [R, 1], f32, kind="ExternalInput").ap()
    lnb_d = nc.dram_tensor("lnbc", [R, 1], f32, kind="ExternalInput").ap()
    b2_d = nc.dram_tensor("b2r", [B_PER_CORE, C], f32, kind="ExternalInput").ap()
    out_d = nc.dram_tensor("out", [B_PER_CORE, C], f32, kind="ExternalOutput").ap()

    NPHASE = B_PER_CORE * H  # 4 phases: (b, h)

    with tile.TileContext(nc) as tc:
        with (
            tc.tile_pool(name="xp", bufs=8) as xp,
            tc.tile_pool(name="cp", bufs=1) as cp,
            tc.tile_pool(name="wp", bufs=1) as wp,
            tc.tile_pool(name="ebp", bufs=2) as ebp,
            tc.tile_pool(name="ps", bufs=1, space="PSUM") as ps,
        ):
            wmT = cp.tile([128, NCHUNK], f32r, tag="wmT")
            nc.gpsimd.dma_start(wmT[:], wmT_d.bitcast(f32r))
            w1t = cp.tile([128, NCHUNK * R], f32, tag="w1t")
            nc.gpsimd.dma_start(w1t[:], w1t_d)
            w2t = cp.tile([R, C], f32, tag="w2t")
            nc.gpsimd.dma_start(w2t[:], w2t_d)
            b1c = cp.tile([R, 1], f32, tag="b1c")
            nc.gpsimd.dma_start(b1c[:], b1_d)
            lngc = cp.tile([R, 1], f32, tag="lngc")
            nc.gpsimd.dma_start(lngc[:], lng_d)
            lnbc = cp.tile([R, 1], f32, tag="lnbc")
            nc.gpsimd.dma_start(lnbc[:], lnb_d)
            b2r = cp.tile([B_PER_CORE, C], f32, tag="b2r")
            nc.gpsimd.dma_start(b2r[:], b2_d)

            # ctx partial columns, col = 16*h + 2*k + b
            ctx_e = wp.tile([128, NPHASE * NCHUNK], f32, tag="ctx_e")
            ctx_u = wp.tile([128, NPHASE * NCHUNK], f32, tag="ctx_u")
            zs = wp.tile([1, NPHASE], f32, tag="zs")

            # PE warm-up / wait absorber (fp32 so any-N is legal)
            dum = ps.tile([1, 1], f32, tag="big1")
            nc.tensor.matmul(dum[:], w1t[:, :1], w1t[:, :1], start=True, stop=True)

            ewarm = wp.tile([1, 1], f32, tag="ewarm")
            nc.scalar.activation(ewarm[:], zs[:, :1], AF.Exp)

            junk = wp.tile([128, 3072], bf16, tag="junk")
            scr = wp.tile([128, 3072], bf16, tag="scr")

            PH = [(0, 3072), (3072, 1024)]   # (offset, size) unequal s-phases
            for b in range(B_PER_CORE):
                for p, (off, SP) in enumerate(PH):
                    xt = []
                    for k in range(NCHUNK):
                        t = xp.tile([128, SP], f32r, tag=f"x{p}")
                        nc.sync.dma_start(
                            t[:],
                            x_d[
                                b, 128 * k : 128 * (k + 1), off : off + SP
                            ].bitcast(f32r),
                        )
                        xt.append(t)

                    big = ps.tile([1, SP], f32, tag=f"big{p}")
                    for k in range(NCHUNK):
                        for j in range(SP // 512):
                            nc.tensor.matmul(
                                big[:, 512 * j : 512 * (j + 1)],
                                wmT[:, k : k + 1],
                                xt[k][:, 512 * j : 512 * (j + 1)],
                                start=(k == 0),
                                stop=(k == NCHUNK - 1),
                            )
                        if p == 0:
                            # uniform part: rowsum of the chunk, as it arrives
                            nc.scalar.activation(
                                junk[:, :SP], xt[k][:].bitcast(f32), AF.Copy,
                                accum_out=ctx_u[:, 16 * p + 2 * k + b : 16 * p + 2 * k + b + 1],
                            )

                    e = ebp.tile([1, SP], f32, tag="e")
                    nc.scalar.activation(
                        e[:], big[:], AF.Exp,
                        accum_out=zs[:, 2 * b + p : 2 * b + p + 1],
                    )
                    eB = ebp.tile([128, SP], f32, tag="eB")
                    nc.gpsimd.partition_broadcast(eB[:], e[:])
                    if p == 1:
                        # small phase: rowsums go after the exp so the exp
                        # fires the moment cm is done (ACT queue is in-order)
                        for k in range(NCHUNK):
                            nc.scalar.activation(
                                junk[:, :SP], xt[k][:].bitcast(f32), AF.Copy,
                                accum_out=ctx_u[:, 16 * p + 2 * k + b : 16 * p + 2 * k + b + 1],
                            )

                    for k in range(NCHUNK):
                        nc.vector.scalar_tensor_tensor(
                            out=scr[:, :SP],
                            in0=xt[k][:].bitcast(f32),
                            scalar=1.0,
                            in1=eB[:],
                            op0=ALU.mult,
                            op1=ALU.mult,
                            accum_out=ctx_e[:, 16 * p + 2 * k + b : 16 * p + 2 * k + b + 1],
                        )

            # ---- combine halves + normalization ----
            zb = wp.tile([1, B_PER_CORE], f32, tag="zb")
            for b in range(B_PER_CORE):
                nc.vector.tensor_add(
                    zb[:, b : b + 1], zs[:, 2 * b : 2 * b + 1], zs[:, 2 * b + 1 : 2 * b + 2]
                )
            zbinv = wp.tile([1, B_PER_CORE], f32, tag="zbinv")
            nc.vector.reciprocal(zbinv[:], zb[:])
            zinv128 = [None] * B_PER_CORE
            for b in range(B_PER_CORE):
                zi = ebp.tile([128, 1], f32, tag="zinv128")
                nc.gpsimd.partition_broadcast(zi[:], zbinv[:, b : b + 1])
                zinv128[b] = zi

            ctxE = wp.tile([128, 2 * NCHUNK], f32, tag="ctxE")
            nc.vector.tensor_add(ctxE[:], ctx_e[:, :16], ctx_e[:, 16:])
            ctxU = wp.tile([128, 2 * NCHUNK], f32, tag="ctxU")
            nc.vector.tensor_add(ctxU[:], ctx_u[:, :16], ctx_u[:, 16:])
            ctxEs = wp.tile([128, 2 * NCHUNK], f32, tag="ctxEs")
            for b in range(B_PER_CORE):
                nc.vector.tensor_scalar(
                    out=ctxEs[:, b :: 2], in0=ctxE[:, b :: 2], scalar1=zinv128[b][:],
                    scalar2=None, op0=ALU.mult,
                )
            ctxA = wp.tile([128, 2 * NCHUNK], f32, tag="ctxA")
            nc.vector.scalar_tensor_tensor(
                out=ctxA[:], in0=ctxU[:], scalar=1.0 / S, in1=ctxEs[:],
                op0=ALU.mult, op1=ALU.add,
            )

            # ---- MLP tail (both batches together) ----
            tps = ps.tile([R, B_PER_CORE], f32, tag="big1")
            for k in range(NCHUNK):
                nc.tensor.matmul(
                    tps[:],
                    w1t[:, R * k : R * (k + 1)],
                    ctxA[:, 2 * k : 2 * k + 2],
                    start=(k == 0),
                    stop=(k == NCHUNK - 1),
                )
            t_sb = wp.tile([R, B_PER_CORE], f32, tag="t_sb")
            nc.vector.tensor_scalar(
                out=t_sb[:], in0=tps[:], scalar1=b1c[:], scalar2=None, op0=ALU.add
            )
            # LayerNorm over r (partition dim) via gpsimd all-reduce
            s1 = wp.tile([R, B_PER_CORE], f32, tag="s1")
            nc.gpsimd.partition_all_reduce(s1[:], t_sb[:], R, bass_isa.ReduceOp.add)
            mu = wp.tile([R, B_PER_CORE], f32, tag="mu")
            nc.vector.tensor_scalar_mul(mu[:], s1[:], 1.0 / R)
            tctr = wp.tile([R, B_PER_CORE], f32, tag="tctr")
            nc.vector.tensor_sub(tctr[:], t_sb[:], mu[:])
            sq = wp.tile([R, B_PER_CORE], f32, tag="sq")
            nc.vector.tensor_mul(sq[:], tctr[:], tctr[:])
            ss = wp.tile([R, B_PER_CORE], f32, tag="ss")
            nc.gpsimd.partition_all_reduce(ss[:], sq[:], R, bass_isa.ReduceOp.add)
            var = wp.tile([R, B_PER_CORE], f32, tag="var")
            nc.vector.tensor_scalar(
                out=var[:], in0=ss[:], scalar1=1.0 / R, scalar2=LN_EPS,
                op0=ALU.mult, op1=ALU.add,
            )
            std = wp.tile([R, B_PER_CORE], f32, tag="std")
            nc.scalar.sqrt(std[:], var[:])
            rstd = wp.tile([R, B_PER_CORE], f32, tag="rstd")
            nc.vector.reciprocal(rstd[:], std[:])
            tn = wp.tile([R, B_PER_CORE], f32, tag="tn")
            nc.vector.tensor_mul(tn[:], tctr[:], rstd[:])
            tg = wp.tile([R, B_PER_CORE], f32, tag="tg")
            nc.vector.tensor_scalar(
                out=tg[:], in0=tn[:], scalar1=lngc[:], scalar2=lnbc[:],
                op0=ALU.mult, op1=ALU.add,
            )
            tr = wp.tile([R, B_PER_CORE], f32, tag="tr")
            nc.vector.tensor_scalar_max(tr[:], tg[:], 0.0)

            ops_ = ps.tile([B_PER_CORE, C], f32, tag="big1")
            for hh in range(C // 512):
                nc.tensor.matmul(
                    ops_[:, 512 * hh : 512 * (hh + 1)],
                    tr[:],
                    w2t[:, 512 * hh : 512 * (hh + 1)],
                    start=True,
                    stop=True,
                )
            out_sb = wp.tile([B_PER_CORE, C], f32, tag="out_sb")
            nc.vector.tensor_add(out_sb[:], ops_[:], b2r[:])
            nc.sync.dma_start(out_d[:], out_sb[:])

    nc.compile()
    return nc


def _prep_inputs(x, wm, w1, b1, ln_g, ln_b, w2, b2):
    x = np.ascontiguousarray(x, dtype=np.float32).reshape(16, C, S)
    wmT = np.ascontiguousarray(wm.astype(np.float32).reshape(NCHUNK, 128).T)
    # w1t[p, 64k+r] = w1[r, 128k+p]
    w1t = np.ascontiguousarray(
        w1.astype(np.float32).reshape(R, NCHUNK, 128).transpose(2, 1, 0).reshape(128, NCHUNK * R)
    )
    w2t = np.ascontiguousarray(w2.astype(np.float32).T)
    b1c = np.ascontiguousarray(b1.astype(np.float32).reshape(R, 1))
    lngc = np.ascontiguousarray(ln_g.astype(np.float32).reshape(R, 1))
    lnbc = np.ascontiguousarray(ln_b.astype(np.float32).reshape(R, 1))
    b2r = np.ascontiguousarray(
        np.broadcast_to(b2.astype(np.float32)[None, :], (B_PER_CORE, C))
    )
    in_maps = []
    for c in range(N_CORES):
        in_maps.append(
            {
                "x": x[B_PER_CORE * c : B_PER_CORE * (c + 1)],
                "wmT": wmT,
                "w1t": w1t,
                "w2t": w2t,
                "b1c": b1c,
                "lngc": lngc,
                "lnbc": lnbc,
                "b2r": b2r,
            }
        )
    return in_maps


def _run(inputs, trace=False, trace_kwargs=None, tmpdir=None):
    from concourse.bass_utils import run_bass_kernel_spmd

    if "nc" not in _CACHE:
        _CACHE["nc"] = _build()
    nc = _CACHE["nc"]
    in_maps = _prep_inputs(
        inputs["x"], inputs["wm"], inputs["w1"], inputs["b1"],
        inputs["ln_g"], inputs["ln_b"], inputs["w2"], inputs["b2"],
    )
    br = run_bass_kernel_spmd(
        nc, in_maps, list(range(N_CORES)), trace=trace,
        trace_kwargs=trace_kwargs or {}, tmpdir=tmpdir,
    )
    out = np.concatenate([np.asarray(r["out"]) for r in br.results], axis=0)
    return out.reshape(16, C, 1, 1).astype(np.float32), br


def kernel(x, wm, bm, w1, b1, ln_g, ln_b, w2, b2):
    inputs = dict(x=x, wm=wm, bm=bm, w1=w1, b1=b1, ln_g=ln_g, ln_b=ln_b, w2=w2, b2=b2)
    out, _ = _run({k: np.asarray(v) for k, v in inputs.items()})
    return out


# revision 19
# speedup vs baseline: 1.1703x; 1.1703x over previous
"""Trainium2 Bass kernel for nn_DGC_Attention (global-context attention block).

Math (per batch b):
    cm[s]   = sum_c x[b,c,s] * wm[c]            (+ bm, which cancels in softmax)
    mask[s] = softmax(cm)[s] + 1/S              (uniform part: softmax of zeros)
    ctx[c]  = sum_s x[b,c,s] * mask[s]
    t       = relu(LN(ctx @ w1.T + b1) * ln_g + ln_b)
    out     = t @ w2.T + b2                     -> [B, C, 1, 1]

Sharding: pure data parallel, batch dim (16) over 8 cores, 2 batches/core.

Key restructuring: no max-subtraction is needed (cm has small range), so
e = exp(cm) is computed per s-HALF and the context sum is assembled from
unnormalized partials:
    ctx[c] = (1/Z) * sum_h sum_{s in h} x[c,s] e[s]  +  (1/S) * sum_s x[c,s]
Z and the 1/S uniform part fold into the tiny MLP stage.  This removes the
global softmax barrier: each (batch, s-half) phase pipelines independently,
so the weighted-pool pass overlaps the DMA stream except for the last half.

Per-core pipeline, per (batch, s-half) phase (x stays fp32/f32r on chip):
    - 8 chunk DMAs [128, 2048] fp32 (HWDGE, 1 MB each).
    - PE: cm partial via f32r matmuls -> [1, 2048] PSUM (4 banks, 2 slots).
    - ACT per chunk: Copy with accum_out -> rowsum column (uniform part),
      runs as chunks arrive (fully hidden).
    - ACT at half end: one Exp over PSUM row, accum_out = Z half.
    - GPSIMD: partition_broadcast e -> eB [128, 2048].
    - DVE per chunk: scalar_tensor_tensor (x * eB, accum) -> ctx_e column.
    - MLP tail: combine halves, scale by 1/Z (via partition-broadcast scalar)
      and 1/S, then matmuls + LayerNorm in transposed layout ([64, 2], LN via
      gpsimd partition_all_reduce); w1/w2 pre-transposed on host.
"""
import numpy as np

B_PER_CORE = 2
N_CORES = 8
C = 1024
S = 4096
H = 2                       # s-halves
SH = S // H                 # 2048
R = 64
NCHUNK = C // 128           # 8 c-chunks
LN_EPS = 1e-5

_CACHE = {}


def _build():
    import concourse.bass as bass
    import concourse.tile as tile
    from concourse import bacc, mybir, bass_isa

    f32 = mybir.dt.float32
    f32r = mybir.dt.float32r
    bf16 = mybir.dt.bfloat16
    AF = mybir.ActivationFunctionType
    ALU = mybir.AluOpType

    nc = bacc.Bacc("TRN2", target_bir_lowering=False, debug=False, num_devices=N_CORES)

    x_d = nc.dram_tensor("x", [B_PER_CORE, C, S], f32, kind="ExternalInput").ap()
    wmT_d = nc.dram_tensor("wmT", [128, NCHUNK], f32, kind="ExternalInput").ap()
    w1t_d = nc.dram_tensor("w1t", [128, NCHUNK * R], f32, kind="ExternalInput").ap()
    w2t_d = nc.dram_tensor("w2t", [R, C], f32, kind="ExternalInput").ap()
    b1_d = nc.dram_tensor("b1c", [R, 1], f32, kind="ExternalInput").ap()
    lng_d = nc.dram_tensor("lngc", [R, 1], f32, kind="ExternalInput").ap()
    lnb_d = nc.dram_tensor("lnbc", [R, 1], f32, kind="ExternalInput").ap()
    b2_d = nc.dram_tensor("b2r", [B_PER_CORE, C], f32, kind="ExternalInput").ap()
    out_d = nc.dram_tensor("out", [B_PER_CORE, C], f32, kind="ExternalOutput").ap()

    NPHASE = B_PER_CORE * H  # 4 phases: (b, h)

    with tile.TileContext(nc) as tc:
        with (
            tc.tile_pool(name="xp", bufs=18) as xp,
            tc.tile_pool(name="cp", bufs=1) as cp,
            tc.tile_pool(name="wp", bufs=1) as wp,
            tc.tile_pool(name="ebp", bufs=2) as ebp,
            tc.tile_pool(name="ps", bufs=2, space="PSUM") as ps,
        ):
            wmT = cp.tile([128, NCHUNK], f32r, tag="wmT")
            nc.gpsimd.dma_start(wmT[:], wmT_d.bitcast(f32r))
            w1t = cp.tile([128, NCHUNK * R], f32, tag="w1t")
            nc.gpsimd.dma_start(w1t[:], w1t_d)
            w2t = cp.tile([R, C], f32, tag="w2t")
            nc.gpsimd.dma_start(w2t[:], w2t_d)
            b1c = cp.tile([R, 1], f32, tag="b1c")
            nc.gpsimd.dma_start(b1c[:], b1_d)
            lngc = cp.tile([R, 1], f32, tag="lngc")
            nc.gpsimd.dma_start(lngc[:], lng_d)
            lnbc = cp.tile([R, 1], f32, tag="lnbc")
            nc.gpsimd.dma_start(lnbc[:], lnb_d)
            b2r = cp.tile([B_PER_CORE, C], f32, tag="b2r")
            nc.gpsimd.dma_start(b2r[:], b2_d)

            # ctx partial columns, col = 16*h + 2*k + b
            ctx_e = wp.tile([128, NPHASE * NCHUNK], f32, tag="ctx_e")
            ctx_u = wp.tile([128, NPHASE * NCHUNK], f32, tag="ctx_u")
            zs = wp.tile([1, NPHASE], f32, tag="zs")

            # PE warm-up / wait absorber (fp32 so any-N is legal)
            dum = ps.tile([1, 1], f32, tag="big")
            nc.tensor.matmul(dum[:], w1t[:, :1], w1t[:, :1], start=True, stop=True)

            ewarm = wp.tile([1, 1], f32, tag="ewarm")
            nc.scalar.activation(ewarm[:], zs[:, :1], AF.Exp)

            junk = wp.tile([128, SH], bf16, tag="junk")
            scr = wp.tile([128, SH], bf16, tag="scr")

            for b in range(B_PER_CORE):
                for h in range(H):
                    xt = []
                    for k in range(NCHUNK):
                        t = xp.tile([128, SH], f32r, tag="x")
                        nc.sync.dma_start(
                            t[:],
                            x_d[
                                b, 128 * k : 128 * (k + 1), SH * h : SH * (h + 1)
                            ].bitcast(f32r),
                        )
                        xt.append(t)

                    big = ps.tile([1, SH], f32, tag="big")
                    for k in range(NCHUNK):
                        for j in range(SH // 512):
                            nc.tensor.matmul(
                                big[:, 512 * j : 512 * (j + 1)],
                                wmT[:, k : k + 1],
                                xt[k][:, 512 * j : 512 * (j + 1)],
                                start=(k == 0),
                                stop=(k == NCHUNK - 1),
                            )
                    e = ebp.tile([1, SH], f32, tag="e")
                    nc.scalar.activation(
                        e[:], big[:], AF.Exp,
                        accum_out=zs[:, 2 * b + h : 2 * b + h + 1],
                    )
                    eB = ebp.tile([128, SH], f32, tag="eB")
                    nc.gpsimd.partition_broadcast(eB[:], e[:])
                    for k in range(NCHUNK):
                        # uniform part: rowsum per chunk, after the exp so the
                        # exp fires the moment cm is done (ACT queue in-order)
                        nc.scalar.activation(
                            junk[:], xt[k][:].bitcast(f32), AF.Copy,
                            accum_out=ctx_u[:, 16 * h + 2 * k + b : 16 * h + 2 * k + b + 1],
                        )

                    for k in range(NCHUNK):
                        nc.vector.scalar_tensor_tensor(
                            out=scr[:],
                            in0=xt[k][:].bitcast(f32),
                            scalar=1.0,
                            in1=eB[:],
                            op0=ALU.mult,
                            op1=ALU.mult,
                            accum_out=ctx_e[:, 16 * h + 2 * k + b : 16 * h + 2 * k + b + 1],
                        )

            # ---- combine halves + normalization ----
            zb = wp.tile([1, B_PER_CORE], f32, tag="zb")
            for b in range(B_PER_CORE):
                nc.vector.tensor_add(
                    zb[:, b : b + 1], zs[:, 2 * b : 2 * b + 1], zs[:, 2 * b + 1 : 2 * b + 2]
                )
            zbinv = wp.tile([1, B_PER_CORE], f32, tag="zbinv")
            nc.vector.reciprocal(zbinv[:], zb[:])
            zinv128 = [None] * B_PER_CORE
            for b in range(B_PER_CORE):
                zi = ebp.tile([128, 1], f32, tag="zinv128")
                nc.gpsimd.partition_broadcast(zi[:], zbinv[:, b : b + 1])
                zinv128[b] = zi

            ctxE = wp.tile([128, 2 * NCHUNK], f32, tag="ctxE")
            nc.vector.tensor_add(ctxE[:], ctx_e[:, :16], ctx_e[:, 16:])
            ctxU = wp.tile([128, 2 * NCHUNK], f32, tag="ctxU")
            nc.vector.tensor_add(ctxU[:], ctx_u[:, :16], ctx_u[:, 16:])
            ctxEs = wp.tile([128, 2 * NCHUNK], f32, tag="ctxEs")
            for b in range(B_PER_CORE):
                nc.vector.tensor_scalar(
                    out=ctxEs[:, b :: 2], in0=ctxE[:, b :: 2], scalar1=zinv128[b][:],
                    scalar2=None, op0=ALU.mult,
                )
            ctxA = wp.tile([128, 2 * NCHUNK], f32, tag="ctxA")
            nc.vector.scalar_tensor_tensor(
                out=ctxA[:], in0=ctxU[:], scalar=1.0 / S, in1=ctxEs[:],
                op0=ALU.mult, op1=ALU.add,
            )

            # ---- MLP tail (both batches together) ----
            tps = ps.tile([R, B_PER_CORE], f32, tag="big")
            for k in range(NCHUNK):
                nc.tensor.matmul(
                    tps[:],
                    w1t[:, R * k : R * (k + 1)],
                    ctxA[:, 2 * k : 2 * k + 2],
                    start=(k == 0),
                    stop=(k == NCHUNK - 1),
                )
            t_sb = wp.tile([R, B_PER_CORE], f32, tag="t_sb")
            nc.vector.tensor_scalar(
                out=t_sb[:], in0=tps[:], scalar1=b1c[:], scalar2=None, op0=ALU.add
            )
            # LayerNorm over r (partition dim) via gpsimd all-reduce
            s1 = wp.tile([R, B_PER_CORE], f32, tag="s1")
            nc.gpsimd.partition_all_reduce(s1[:], t_sb[:], R, bass_isa.ReduceOp.add)
            mu = wp.tile([R, B_PER_CORE], f32, tag="mu")
            nc.vector.tensor_scalar_mul(mu[:], s1[:], 1.0 / R)
            tctr = wp.tile([R, B_PER_CORE], f32, tag="tctr")
            nc.vector.tensor_sub(tctr[:], t_sb[:], mu[:])
            sq = wp.tile([R, B_PER_CORE], f32, tag="sq")
            nc.vector.tensor_mul(sq[:], tctr[:], tctr[:])
            ss = wp.tile([R, B_PER_CORE], f32, tag="ss")
            nc.gpsimd.partition_all_reduce(ss[:], sq[:], R, bass_isa.ReduceOp.add)
            var = wp.tile([R, B_PER_CORE], f32, tag="var")
            nc.vector.tensor_scalar(
                out=var[:], in0=ss[:], scalar1=1.0 / R, scalar2=LN_EPS,
                op0=ALU.mult, op1=ALU.add,
            )
            std = wp.tile([R, B_PER_CORE], f32, tag="std")
            nc.scalar.sqrt(std[:], var[:])
            rstd = wp.tile([R, B_PER_CORE], f32, tag="rstd")
            nc.vector.reciprocal(rstd[:], std[:])
            tn = wp.tile([R, B_PER_CORE], f32, tag="tn")
            nc.vector.tensor_mul(tn[:], tctr[:], rstd[:])
            tg = wp.tile([R, B_PER_CORE], f32, tag="tg")
            nc.vector.tensor_scalar(
                out=tg[:], in0=tn[:], scalar1=lngc[:], scalar2=lnbc[:],
                op0=ALU.mult, op1=ALU.add,
            )
            tr = wp.tile([R, B_PER_CORE], f32, tag="tr")
            nc.vector.tensor_scalar_max(tr[:], tg[:], 0.0)

            ops_ = ps.tile([B_PER_CORE, C], f32, tag="big")
            for hh in range(C // 512):
                nc.tensor.matmul(
                    ops_[:, 512 * hh : 512 * (hh + 1)],
                    tr[:],
                    w2t[:, 512 * hh : 512 * (hh + 1)],
                    start=True,
                    stop=True,
                )
            out_sb = wp.tile([B_PER_CORE, C], f32, tag="out_sb")
            nc.vector.tensor_add(out_sb[:], ops_[:], b2r[:])
            nc.sync.dma_start(out_d[:], out_sb[:])

    nc.compile()
    return nc


def _prep_inputs(x, wm, w1, b1, ln_g, ln_b, w2, b2):
    x = np.ascontiguousarray(x, dtype=np.float32).reshape(16, C, S)
    wmT = np.ascontiguousarray(wm.astype(np.float32).reshape(NCHUNK, 128).T)
    # w1t[p, 64k+r] = w1[r, 128k+p]
    w1t = np.ascontiguousarray(
        w1.astype(np.float32).reshape(R, NCHUNK, 128).transpose(2, 1, 0).reshape(128, NCHUNK * R)
    )
    w2t = np.ascontiguousarray(w2.astype(np.float32).T)
    b1c = np.ascontiguousarray(b1.astype(np.float32).reshape(R, 1))
    lngc = np.ascontiguousarray(ln_g.astype(np.float32).reshape(R, 1))
    lnbc = np.ascontiguousarray(ln_b.astype(np.float32).reshape(R, 1))
    b2r = np.ascontiguousarray(
        np.broadcast_to(b2.astype(np.float32)[None, :], (B_PER_CORE, C))
    )
    in_maps = []
    for c in range(N_CORES):
        in_maps.append(
            {
                "x": x[B_PER_CORE * c : B_PER_CORE * (c + 1)],
                "wmT": wmT,
                "w1t": w1t,
                "w2t": w2t,
                "b1c": b1c,
                "lngc": lngc,
                "lnbc": lnbc,
                "b2r": b2r,
            }
        )
    return in_maps


def _run(inputs, trace=False, trace_kwargs=None, tmpdir=None):
    from concourse.bass_utils import run_bass_kernel_spmd

    if "nc" not in _CACHE:
        _CACHE["nc"] = _build()
    nc = _CACHE["nc"]
    in_maps = _prep_inputs(
        inputs["x"], inputs["wm"], inputs["w1"], inputs["b1"],
        inputs["ln_g"], inputs["ln_b"], inputs["w2"], inputs["b2"],
    )
    br = run_bass_kernel_spmd(
        nc, in_maps, list(range(N_CORES)), trace=trace,
        trace_kwargs=trace_kwargs or {}, tmpdir=tmpdir,
    )
    out = np.concatenate([np.asarray(r["out"]) for r in br.results], axis=0)
    return out.reshape(16, C, 1, 1).astype(np.float32), br


def kernel(x, wm, bm, w1, b1, ln_g, ln_b, w2, b2):
    inputs = dict(x=x, wm=wm, bm=bm, w1=w1, b1=b1, ln_g=ln_g, ln_b=ln_b, w2=w2, b2=b2)
    out, _ = _run({k: np.asarray(v) for k, v in inputs.items()})
    return out
